# revision 3
# baseline (speedup 1.0000x reference)
"""ComplexAttention Trainium2 kernel (8 NeuronCores, SPMD).

Math: the reference "complex" attention reduces exactly to standard
single-head attention with head_dim 2D=2048 and scale 1/sqrt(D):
  Re(q . conj(k)) over interleaved (r,i) pairs == full dot product q.k
  interleave(o_r, o_i)                         == softmax_w @ v_full

Algebraic fusion (host-side, weights only):
  logits[s,t] = hs[s] @ A @ hs[t]^T + (hs @ u2)[t]   (+ per-row const, dropped)
      A  = Wq^T Wk / sqrt(D),  u2 = Wk^T bq / sqrt(D)
  out[s]  = ((P_un @ hs) @ MT)[s] / sumexp[s] + biasf
      MT = Wv^T Wo^T,  biasf = bo + Wo bv
The (P_un @ hs) @ MT association (vs P_un @ (hs @ MT)) cuts tail FLOPs 25%
and removes the duplicated hs @ MT work across the two cores of a batch.
u2 is folded into qhat as a per-partition bias at PSUM-copy time, so no
separate termt matmuls are needed.

All matmul operands are bf16 (PSUM accumulates fp32): on real TRN2 silicon
fp32/f32r matmuls stream at 4 cycles/row vs bf16's 1, so this is ~4x on the
PE-bound part. rel-err budget is 2e-2; bf16 lands ~2e-3.

Sharding: 8 cores = 4 batches x 2 query-halves. Each core gets its batch's
hidden_states rotated so its 1024 query rows are rows 0:1024; keys/values
span the full (rotated) sequence -- softmax over keys is permutation
invariant, so rotation is exact.
"""

import math
import os
import time

import numpy as np

B, S, D = 4, 2048, 1024
P = 128
NCORES = 8
SQ = S // 2          # query rows per core
DC = D // P          # 8  d-chunks
TT = S // P          # 16 t-tiles
ST = SQ // P         # 8  s-tiles
FH = 2               # f output halves
FW = D // FH         # 512

_CACHE = {}
LAST_TIMING = {}


def _emit(nc, tc, tile, mybir, make_identity, aps):
    f32 = mybir.dt.float32
    bf16 = mybir.dt.bfloat16
    Exp = mybir.ActivationFunctionType.Exp
    Copy = mybir.ActivationFunctionType.Copy
    Ident = mybir.ActivationFunctionType.Identity

    x, A, MT, u2s, onesc, biasb, y = (
        aps["x"], aps["A"], aps["MT"], aps["u2s"], aps["onesc"],
        aps["biasb"], aps["y"],
    )

    with (
        tc.tile_pool(name="persist", bufs=1) as persist,
        tc.tile_pool(name="psum_mm", bufs=4, space="PSUM") as psum_mm,
        tc.tile_pool(name="psum_sm", bufs=2, space="PSUM") as psum_sm,
        tc.tile_pool(name="outp", bufs=3) as outp,
    ):
        x_td = persist.tile([P, TT, D], bf16)      # 32 KB/p  hs[t, d]
        hsT = persist.tile([P, DC, S], bf16)       # 32 KB/p  hs[d, t]
        qhatT = persist.tile([P, DC, SQ], bf16)    # 16 KB/p
        expT = persist.tile([P, TT, SQ], bf16)     # 32 KB/p
        g1T = persist.tile([P, DC, SQ], bf16)      # 16 KB/p
        aT = persist.tile([P, DC, D], bf16)        # 16 KB/p  A[d, d']
        mT = persist.tile([P, DC, D], bf16)        # 16 KB/p  MT[d, f]
        accS = persist.tile([P, SQ], f32)          # 4 KB/p
        identity = persist.tile([P, P], bf16)
        recipS = persist.tile([P, ST], f32)        # striped 1/sumexp
        u2s_sb = persist.tile([P, DC], f32)
        onesc_sb = persist.tile([P, 2], f32)
        biasb_sb = persist.tile([P, D], f32)       # 4 KB/p

        make_identity(nc, identity)

        # ---- all input DMAs up front: x tiles first (gate everything),
        # then A / MT / small constants (overlap with transpose phase) ----
        for t16 in range(TT):
            nc.sync.dma_start(x_td[:, t16, :], x[t16 * P:(t16 + 1) * P, :])
        nc.sync.dma_start(aT, A.rearrange("(o p) n -> p o n", p=P))
        nc.sync.dma_start(mT, MT.rearrange("(o p) f -> p o f", p=P))
        nc.sync.dma_start(u2s_sb, u2s)
        nc.sync.dma_start(onesc_sb, onesc)
        nc.sync.dma_start(biasb_sb, biasb)

        # ---- P0: transpose hs -> hsT via PE (bf16, 1 cyc/row) ----
        for t16 in range(TT):
            for dc in range(DC):
                pt = psum_mm.tile([P, 512], f32, tag="mm",
                                  name="mm_ps")[:, :P].bitcast(bf16)[:, :P]
                nc.tensor.transpose(pt, x_td[:, t16, dc * P:(dc + 1) * P],
                                    identity)
                if dc % 2 == 0:
                    nc.vector.tensor_copy(
                        out=hsT[:, dc, t16 * P:(t16 + 1) * P], in_=pt)
                else:
                    nc.scalar.copy(
                        out=hsT[:, dc, t16 * P:(t16 + 1) * P], in_=pt)

        # ---- P1: qhatT[d', s] = sum_d A[d, d'] hsT[d, s(q)] + u2[d'] ----
        for dpt in range(DC):
            for sh in range(2):
                ps = psum_mm.tile([P, 512], f32, tag="mm", name="mm_ps")
                for dc in range(DC):
                    nc.tensor.matmul(
                        ps,
                        lhsT=aT[:, dc, dpt * P:(dpt + 1) * P],
                        rhs=hsT[:, dc, sh * 512:(sh + 1) * 512],
                        start=(dc == 0),
                        stop=(dc == DC - 1),
                    )
                # copy-cast to bf16 with the folded u2 bias (per-partition)
                nc.scalar.activation(
                    qhatT[:, dpt, sh * 512:(sh + 1) * 512], ps, Ident,
                    bias=u2s_sb[:, dpt:dpt + 1])

        # ---- P2: scoresT[t, s] -> exp; DVE-accumulate sumexp over t ----
        for tt in range(TT):
            for sh in range(2):
                ps = psum_mm.tile([P, 512], f32, tag="mm", name="mm_ps")
                for dc in range(DC):
                    nc.tensor.matmul(
                        ps,
                        lhsT=hsT[:, dc, tt * P:(tt + 1) * P],
                        rhs=qhatT[:, dc, sh * 512:(sh + 1) * 512],
                        start=(dc == 0),
                        stop=(dc == DC - 1),
                    )
                nc.scalar.activation(
                    expT[:, tt, sh * 512:(sh + 1) * 512], ps, Exp)
            if tt == 0:
                nc.vector.tensor_copy(out=accS, in_=expT[:, 0, :])
            else:
                nc.vector.tensor_add(out=accS, in0=accS, in1=expT[:, tt, :])

        # striped sumexp: recipS[p, st] = 1 / sum_t expT[t, st*128+p]
        for st in range(ST):
            sp = psum_sm.tile([P, 2], f32, tag="sm", name="sm_ps")
            nc.tensor.matmul(sp, lhsT=accS[:, st * P:(st + 1) * P],
                             rhs=onesc_sb, start=True, stop=True)
            nc.vector.reciprocal(recipS[:, st:st + 1], sp[:, 0:1])

        # ---- P3: g1T[d, s] = sum_t hs[t, d] expT[t, s] ----
        for sh in range(2):
            for dc in range(DC):
                ps = psum_mm.tile([P, 512], f32, tag="mm", name="mm_ps")
                for tt in range(TT):
                    nc.tensor.matmul(
                        ps,
                        lhsT=x_td[:, tt, dc * P:(dc + 1) * P],
                        rhs=expT[:, tt, sh * 512:(sh + 1) * 512],
                        start=(tt == 0),
                        stop=(tt == TT - 1),
                    )
                if dc % 2 == 0:
                    nc.vector.tensor_copy(
                        out=g1T[:, dc, sh * 512:(sh + 1) * 512], in_=ps)
                else:
                    nc.scalar.copy(
                        out=g1T[:, dc, sh * 512:(sh + 1) * 512], in_=ps)

        # ---- P4: out[s, f] = (sum_d g1T[d,s] MT[d,f]) / sumexp[s] + biasf ----
        for q in range(FH):
            for st in range(ST):
                gp = psum_mm.tile([P, 512], f32, tag="mm", name="mm_ps")
                for dc in range(DC):
                    nc.tensor.matmul(
                        gp,
                        lhsT=g1T[:, dc, st * P:(st + 1) * P],
                        rhs=mT[:, dc, q * FW:(q + 1) * FW],
                        start=(dc == 0),
                        stop=(dc == DC - 1),
                    )
                ot = outp.tile([P, FW], f32, tag="ot", name="ot")
                nc.scalar.activation(ot, gp, Copy, scale=recipS[:, st:st + 1])
                nc.vector.tensor_add(
                    out=ot, in0=ot, in1=biasb_sb[:, q * FW:(q + 1) * FW])
                nc.sync.dma_start(
                    y[st * P:(st + 1) * P, q * FW:(q + 1) * FW], ot)


def _build():
    reps = int(os.environ.get("CPLX_BENCH_REPS", "1"))
    key = ("nc", reps)
    if key in _CACHE:
        return _CACHE[key]
    import concourse.bass as bass  # noqa: F401
    import concourse.tile as tile
    import concourse.mybir as mybir
    from concourse import bacc
    from concourse.masks import make_identity

    f32 = mybir.dt.float32
    bf16 = mybir.dt.bfloat16
    nc = bacc.Bacc("TRN2", target_bir_lowering=False, debug=False,
                   enable_asserts=False, num_devices=NCORES)
    aps = {
        "x": nc.dram_tensor("x", [S, D], bf16, kind="ExternalInput").ap(),
        "A": nc.dram_tensor("A", [D, D], bf16, kind="ExternalInput").ap(),
        "MT": nc.dram_tensor("MT", [D, D], bf16, kind="ExternalInput").ap(),
        "u2s": nc.dram_tensor("u2s", [P, DC], f32, kind="ExternalInput").ap(),
        "onesc": nc.dram_tensor("onesc", [P, 2], f32,
                                kind="ExternalInput").ap(),
        "biasb": nc.dram_tensor("biasb", [P, D], f32, kind="ExternalInput").ap(),
        "y": nc.dram_tensor("y", [SQ, D], f32, kind="ExternalOutput").ap(),
    }
    with tile.TileContext(nc) as tc:
        for _ in range(reps):
            _emit(nc, tc, tile, mybir, make_identity, aps)
    nc.compile()
    _CACHE[key] = nc
    return nc


def _host_prep(inputs):
    import ml_dtypes
    bf16 = ml_dtypes.bfloat16

    hs = np.asarray(inputs["hidden_states"], dtype=np.float32)
    Wq = np.asarray(inputs["Wq"], dtype=np.float64)
    bq = np.asarray(inputs["bq"], dtype=np.float64)
    Wk = np.asarray(inputs["Wk"], dtype=np.float64)
    Wv = np.asarray(inputs["Wv"], dtype=np.float64)
    bv = np.asarray(inputs["bv"], dtype=np.float64)
    Wo = np.asarray(inputs["Wo"], dtype=np.float64)
    bo = np.asarray(inputs["bo"], dtype=np.float64)

    scale = 1.0 / math.sqrt(D)
    A = ((Wq.T @ Wk) * scale).astype(bf16)                  # [d, d']
    u2 = ((Wk.T @ bq) * scale).astype(np.float32)           # [d']
    MT = (Wv.T @ Wo.T).astype(bf16)                         # [d, f]
    biasf = (bo + Wo @ bv).astype(np.float32)               # [f]

    u2s = np.ascontiguousarray(u2.reshape(DC, P).T)         # [128, 8] striped
    onesc = np.ones((P, 2), dtype=np.float32)
    biasb = np.ascontiguousarray(
        np.broadcast_to(biasf[None, :], (P, D)))            # [128, 1024]

    in_maps = []
    for core in range(NCORES):
        b, half = core // 2, core % 2
        if half == 0:
            xc = hs[b]
        else:
            xc = np.concatenate([hs[b, SQ:], hs[b, :SQ]], axis=0)
        in_maps.append({
            "x": np.ascontiguousarray(xc.astype(bf16)),
            "A": A,
            "MT": MT,
            "u2s": u2s,
            "onesc": onesc,
            "biasb": biasb,
        })
    return in_maps


def _make_runner(nc, in_maps):
    """Persistent jitted SPMD runner (mirrors bass2jax.run_bass_via_pjrt)."""
    import jax
    import numpy as np
    from jax.experimental.shard_map import shard_map
    from jax.sharding import Mesh, PartitionSpec
    import concourse.mybir as mybir
    from concourse import bass2jax

    bass2jax.install_neuronx_cc_hook()
    partition_name = (
        nc.partition_id_tensor.name if nc.partition_id_tensor else None)

    in_names, out_names, out_avals, zero_outs = [], [], [], []
    for alloc in nc.m.functions[0].allocations:
        if not isinstance(alloc, mybir.MemoryLocationSet):
            continue
        name = alloc.memorylocations[0].name
        if alloc.kind == "ExternalInput":
            if name != partition_name:
                in_names.append(name)
        elif alloc.kind == "ExternalOutput":
            np_dt = mybir.dt.np(alloc.dtype)
            out_names.append(name)
            out_avals.append(
                jax.core.ShapedArray(tuple(alloc.tensor_shape), np_dt))
            zero_outs.append(
                np.zeros(tuple(alloc.tensor_shape), np_dt))

    n_params = len(in_names)
    n_outs = len(out_avals)
    all_in_names = in_names + out_names
    if partition_name is not None:
        all_in_names = all_in_names + [partition_name]

    def _body(*args):
        operands = list(args)
        if partition_name is not None:
            operands.append(bass2jax.partition_id_tensor())
        outs = bass2jax._bass_exec_p.bind(
            *operands,
            out_avals=tuple(out_avals),
            in_names=tuple(all_in_names),
            out_names=tuple(out_names),
            lowering_input_output_aliases=(),
            sim_require_finite=True,
            sim_require_nnan=True,
            nc=nc,
        )
        return tuple(outs)

    devices = jax.devices()[:NCORES]
    mesh = Mesh(np.asarray(devices), ("core",))
    in_specs = (PartitionSpec("core"),) * (n_params + n_outs)
    out_specs = (PartitionSpec("core"),) * n_outs
    sharded = jax.jit(
        shard_map(_body, mesh=mesh, in_specs=in_specs, out_specs=out_specs,
                  check_rep=False),
        keep_unused=True,
    )

    concat_in = [
        np.concatenate([in_maps[c][nm] for c in range(NCORES)], axis=0)
        for nm in in_names
    ]
    concat_zeros = [
        np.zeros((NCORES * z.shape[0], *z.shape[1:]), z.dtype)
        for z in zero_outs
    ]
    from jax.sharding import NamedSharding
    sharding = NamedSharding(mesh, PartitionSpec("core"))
    args = [jax.device_put(a, sharding)
            for a in [*concat_in, *concat_zeros]]
    jax.block_until_ready(args)

    def run():
        out = sharded(*args)
        jax.block_until_ready(out)
        return out

    def run_queued(n):
        # queue n executions back-to-back, block once: amortizes the
        # multi-ms axon dispatch overhead so the marginal cost per exec
        # approaches true device time
        o = None
        t0 = time.perf_counter()
        for _ in range(n):
            o = sharded(*args)
        jax.block_until_ready(o)
        return time.perf_counter() - t0

    run.queued = run_queued
    return run, out_names, out_avals


def kernel(**inputs):
    in_maps = _host_prep(inputs)
    nc = _build()
    run, out_names, out_avals = _make_runner(nc, in_maps)

    t0 = time.perf_counter()
    out_arrs = run()  # first call compiles
    t1 = time.perf_counter()

    n_timed = int(os.environ.get("CPLX_TIMED_ITERS", "0"))
    times = []
    for _ in range(n_timed):
        ts = time.perf_counter()
        run()
        times.append(time.perf_counter() - ts)
    marginal = None
    if n_timed:
        # slope over queued batches: subtracts fixed dispatch overhead
        lo, hi = 8, 128
        t_lo = min(run.queued(lo) for _ in range(2))
        t_hi = min(run.queued(hi) for _ in range(2))
        marginal = (t_hi - t_lo) / (hi - lo)
    LAST_TIMING.clear()
    LAST_TIMING.update({
        "first_call_s": t1 - t0,
        "timed_iters_s": times,
        "best_iter_s": min(times) if times else None,
        "marginal_exec_s": marginal,
    })

    yi = out_names.index("y")
    ys = np.asarray(out_arrs[yi]).reshape(NCORES, SQ, D)

    out = np.empty((B, S, D), dtype=np.float32)
    for core in range(NCORES):
        b, half = core // 2, core % 2
        out[b, half * SQ:(half + 1) * SQ, :] = ys[core]
    return out


# revision 10
# speedup vs baseline: 5.7346x; 5.7346x over previous
"""ComplexAttention Trainium2 kernel (8 NeuronCores, SPMD).

Math: the reference "complex" attention reduces exactly to standard
single-head attention with head_dim 2D=2048 and scale 1/sqrt(D):
  Re(q . conj(k)) over interleaved (r,i) pairs == full dot product q.k
  interleave(o_r, o_i)                         == softmax_w @ v_full

Algebraic fusion (host-side, weights only):
  logits[s,t] = hs[s] @ A @ hs[t]^T + (hs @ u2)[t]   (+ per-row const, dropped)
      A  = Wq^T Wk / sqrt(D),  u2 = Wk^T bq / sqrt(D)
  out[s]  = ((P_un @ hs) @ MT)[s] / sumexp[s] + biasf
      MT = Wv^T Wo^T,  biasf = bo + Wo bv
The (P_un @ hs) @ MT association (vs P_un @ (hs @ MT)) cuts tail FLOPs 25%
and removes the duplicated hs @ MT work across the two cores of a batch.
u2 is folded into qhat as a per-partition bias at PSUM-copy time, so no
separate termt matmuls are needed.

All matmul operands are bf16 (PSUM accumulates fp32): on real TRN2 silicon
fp32/f32r matmuls stream at 4 cycles/row vs bf16's 1, so this is ~4x on the
PE-bound part. rel-err budget is 2e-2; bf16 lands ~2e-3.

Sharding: 8 cores = 4 batches x 2 query-halves. Each core gets its batch's
hidden_states rotated so its 1024 query rows are rows 0:1024; keys/values
span the full (rotated) sequence -- softmax over keys is permutation
invariant, so rotation is exact.
"""

import math
import os
import time

import numpy as np

B, S, D = 4, 2048, 1024
P = 128
NCORES = 8
SQ = S // 2          # query rows per core
DC = D // P          # 8  d-chunks
TT = S // P          # 16 t-tiles
ST = SQ // P         # 8  s-tiles
FH = 2               # f output halves
FW = D // FH         # 512

_CACHE = {}
LAST_TIMING = {}


def _emit(nc, tc, tile, mybir, make_identity, aps):
    f32 = mybir.dt.float32
    bf16 = mybir.dt.bfloat16
    Exp = mybir.ActivationFunctionType.Exp
    Copy = mybir.ActivationFunctionType.Copy
    Ident = mybir.ActivationFunctionType.Identity

    x, A, MT, u2s, onesc, biasb, y = (
        aps["x"], aps["A"], aps["MT"], aps["u2s"], aps["onesc"],
        aps["biasb"], aps["y"],
    )

    with (
        tc.tile_pool(name="persist", bufs=1) as persist,
        tc.tile_pool(name="psum_mm", bufs=6, space="PSUM") as psum_mm,
        tc.tile_pool(name="psum_sm", bufs=2, space="PSUM") as psum_sm,
        tc.tile_pool(name="outp", bufs=3) as outp,
    ):
        x_td = persist.tile([P, TT, D], bf16)      # 32 KB/p  hs[t, d]
        hsT = persist.tile([P, DC, S], bf16)       # 32 KB/p  hs[d, t]
        qhatT = persist.tile([P, DC, SQ], bf16)    # 16 KB/p
        expT = persist.tile([P, TT, SQ], bf16)     # 32 KB/p
        g1T = persist.tile([P, DC, SQ], bf16)      # 16 KB/p
        aT = persist.tile([P, DC, D], bf16)        # 16 KB/p  A[d, d']
        mT = persist.tile([P, DC, D], bf16)        # 16 KB/p  MT[d, f]
        accS = persist.tile([P, SQ], f32)          # 4 KB/p
        identity = persist.tile([P, P], bf16)
        recipS = persist.tile([P, ST], f32)        # striped 1/sumexp
        u2s_sb = persist.tile([P, DC], f32)
        onesc_sb = persist.tile([P, 2], f32)
        biasb_sb = persist.tile([P, D], f32)       # 4 KB/p

        make_identity(nc, identity)

        # ---- all input DMAs up front, in consumption order. x in 4-tile
        # batches (one dma_start has ~1us fixed cost; x_td[p,o,d] matches a
        # rearranged contiguous row-block load) ----
        for xb in range(4):
            nc.sync.dma_start(
                x_td[:, 4 * xb:4 * (xb + 1), :],
                x[4 * xb * P:4 * (xb + 1) * P, :].rearrange(
                    "(o p) d -> p o d", p=P))
            if xb == 1:
                nc.sync.dma_start(aT, A.rearrange("(o p) n -> p o n", p=P))
        nc.sync.dma_start(u2s_sb, u2s)
        nc.sync.dma_start(mT, MT.rearrange("(o p) f -> p o f", p=P))
        nc.sync.dma_start(onesc_sb, onesc)
        nc.sync.dma_start(biasb_sb, biasb)

        # ---- P0: transpose hs -> hsT via PE (bf16, 1 cyc/row); 4 chunk
        # transposes share one PSUM tile -> one 512-wide copy out ----
        ncpy = 0
        for t16 in range(TT):
            for dq in range(DC // 4):
                pt = psum_mm.tile([P, 512], f32, tag="mm",
                                  name="mm_ps").bitcast(bf16)[:, :512]
                for k in range(4):
                    nc.tensor.transpose(
                        pt[:, k * P:(k + 1) * P],
                        x_td[:, t16, (4 * dq + k) * P:(4 * dq + k + 1) * P],
                        identity)
                dst = hsT[:, 4 * dq:4 * dq + 4, t16 * P:(t16 + 1) * P]
                src = pt.rearrange("p (o n) -> p o n", o=4)
                if ncpy % 2 == 0:
                    nc.vector.tensor_copy(out=dst, in_=src)
                else:
                    nc.scalar.copy(out=dst, in_=src)
                ncpy += 1

        # ---- P1: qhatT[d', s] = sum_d A[d, d'] hsT[d, s(q)] + u2[d'] ----
        for dpt in range(DC):
            for sh in range(2):
                ps = psum_mm.tile([P, 512], f32, tag="mm", name="mm_ps")
                for dc in range(DC):
                    nc.tensor.matmul(
                        ps,
                        lhsT=aT[:, dc, dpt * P:(dpt + 1) * P],
                        rhs=hsT[:, dc, sh * 512:(sh + 1) * 512],
                        start=(dc == 0),
                        stop=(dc == DC - 1),
                    )
                # copy-cast to bf16 with the folded u2 bias (per-partition)
                nc.scalar.activation(
                    qhatT[:, dpt, sh * 512:(sh + 1) * 512], ps, Ident,
                    bias=u2s_sb[:, dpt:dpt + 1])

        # ---- P2: scoresT[t, s] -> exp; DVE-accumulate sumexp over t ----
        for tt in range(TT):
            for sh in range(2):
                ps = psum_mm.tile([P, 512], f32, tag="mm", name="mm_ps")
                for dc in range(DC):
                    nc.tensor.matmul(
                        ps,
                        lhsT=hsT[:, dc, tt * P:(tt + 1) * P],
                        rhs=qhatT[:, dc, sh * 512:(sh + 1) * 512],
                        start=(dc == 0),
                        stop=(dc == DC - 1),
                    )
                nc.scalar.activation(
                    expT[:, tt, sh * 512:(sh + 1) * 512], ps, Exp)
            if tt == 0:
                nc.vector.tensor_copy(out=accS, in_=expT[:, 0, :])
            else:
                nc.vector.tensor_add(out=accS, in0=accS, in1=expT[:, tt, :])

        # striped sumexp: recipS[p, st] = 1 / sum_t expT[t, st*128+p]
        for st in range(ST):
            sp = psum_sm.tile([P, 2], f32, tag="sm", name="sm_ps")
            nc.tensor.matmul(sp, lhsT=accS[:, st * P:(st + 1) * P],
                             rhs=onesc_sb, start=True, stop=True)
            nc.vector.reciprocal(recipS[:, st:st + 1], sp[:, 0:1])

        # ---- P3: g1T[d, s] = sum_t hs[t, d] expT[t, s] ----
        for sh in range(2):
            for dc in range(DC):
                ps = psum_mm.tile([P, 512], f32, tag="mm", name="mm_ps")
                for tt in range(TT):
                    nc.tensor.matmul(
                        ps,
                        lhsT=x_td[:, tt, dc * P:(dc + 1) * P],
                        rhs=expT[:, tt, sh * 512:(sh + 1) * 512],
                        start=(tt == 0),
                        stop=(tt == TT - 1),
                    )
                if dc % 2 == 0:
                    nc.vector.tensor_copy(
                        out=g1T[:, dc, sh * 512:(sh + 1) * 512], in_=ps)
                else:
                    nc.scalar.copy(
                        out=g1T[:, dc, sh * 512:(sh + 1) * 512], in_=ps)

        # ---- P4: out[s, f] = (sum_d g1T[d,s] MT[d,f]) / sumexp[s] + biasf ----
        # stage 4 s-tiles per f-half into one SBUF tile -> 1 batched DMA
        for q in range(FH):
            for sq in range(ST // 4):
                oquad = outp.tile([P, 4, FW], f32, tag="ot", name="ot")
                for k in range(4):
                    st = 4 * sq + k
                    gp = psum_mm.tile([P, 512], f32, tag="mm", name="mm_ps")
                    for dc in range(DC):
                        nc.tensor.matmul(
                            gp,
                            lhsT=g1T[:, dc, st * P:(st + 1) * P],
                            rhs=mT[:, dc, q * FW:(q + 1) * FW],
                            start=(dc == 0),
                            stop=(dc == DC - 1),
                        )
                    nc.scalar.activation(oquad[:, k, :], gp, Copy,
                                         scale=recipS[:, st:st + 1])
                    nc.vector.tensor_add(
                        out=oquad[:, k, :], in0=oquad[:, k, :],
                        in1=biasb_sb[:, q * FW:(q + 1) * FW])
                nc.sync.dma_start(
                    y[4 * sq * P:4 * (sq + 1) * P,
                      q * FW:(q + 1) * FW].rearrange("(o p) f -> p o f", p=P),
                    oquad)


def _build():
    reps = int(os.environ.get("CPLX_BENCH_REPS", "1"))
    key = ("nc", reps)
    if key in _CACHE:
        return _CACHE[key]
    import concourse.bass as bass  # noqa: F401
    import concourse.tile as tile
    import concourse.mybir as mybir
    from concourse import bacc
    from concourse.masks import make_identity

    f32 = mybir.dt.float32
    bf16 = mybir.dt.bfloat16
    nc = bacc.Bacc("TRN2", target_bir_lowering=False, debug=False,
                   enable_asserts=False, num_devices=NCORES)
    aps = {
        "x": nc.dram_tensor("x", [S, D], bf16, kind="ExternalInput").ap(),
        "A": nc.dram_tensor("A", [D, D], bf16, kind="ExternalInput").ap(),
        "MT": nc.dram_tensor("MT", [D, D], bf16, kind="ExternalInput").ap(),
        "u2s": nc.dram_tensor("u2s", [P, DC], f32, kind="ExternalInput").ap(),
        "onesc": nc.dram_tensor("onesc", [P, 2], f32,
                                kind="ExternalInput").ap(),
        "biasb": nc.dram_tensor("biasb", [P, D], f32, kind="ExternalInput").ap(),
        "y": nc.dram_tensor("y", [SQ, D], f32, kind="ExternalOutput").ap(),
    }
    with tile.TileContext(nc) as tc:
        for _ in range(reps):
            _emit(nc, tc, tile, mybir, make_identity, aps)
    nc.compile()
    _CACHE[key] = nc
    return nc


def _host_prep(inputs):
    import ml_dtypes
    bf16 = ml_dtypes.bfloat16

    hs = np.asarray(inputs["hidden_states"], dtype=np.float32)
    Wq = np.asarray(inputs["Wq"], dtype=np.float64)
    bq = np.asarray(inputs["bq"], dtype=np.float64)
    Wk = np.asarray(inputs["Wk"], dtype=np.float64)
    Wv = np.asarray(inputs["Wv"], dtype=np.float64)
    bv = np.asarray(inputs["bv"], dtype=np.float64)
    Wo = np.asarray(inputs["Wo"], dtype=np.float64)
    bo = np.asarray(inputs["bo"], dtype=np.float64)

    scale = 1.0 / math.sqrt(D)
    A = ((Wq.T @ Wk) * scale).astype(bf16)                  # [d, d']
    u2 = ((Wk.T @ bq) * scale).astype(np.float32)           # [d']
    MT = (Wv.T @ Wo.T).astype(bf16)                         # [d, f]
    biasf = (bo + Wo @ bv).astype(np.float32)               # [f]

    u2s = np.ascontiguousarray(u2.reshape(DC, P).T)         # [128, 8] striped
    onesc = np.ones((P, 2), dtype=np.float32)
    biasb = np.ascontiguousarray(
        np.broadcast_to(biasf[None, :], (P, D)))            # [128, 1024]

    in_maps = []
    for core in range(NCORES):
        b, half = core // 2, core % 2
        if half == 0:
            xc = hs[b]
        else:
            xc = np.concatenate([hs[b, SQ:], hs[b, :SQ]], axis=0)
        in_maps.append({
            "x": np.ascontiguousarray(xc.astype(bf16)),
            "A": A,
            "MT": MT,
            "u2s": u2s,
            "onesc": onesc,
            "biasb": biasb,
        })
    return in_maps


def _make_runner(nc, in_maps):
    """Persistent jitted SPMD runner (mirrors bass2jax.run_bass_via_pjrt)."""
    import jax
    import numpy as np
    from jax.experimental.shard_map import shard_map
    from jax.sharding import Mesh, PartitionSpec
    import concourse.mybir as mybir
    from concourse import bass2jax

    bass2jax.install_neuronx_cc_hook()
    partition_name = (
        nc.partition_id_tensor.name if nc.partition_id_tensor else None)

    in_names, out_names, out_avals, zero_outs = [], [], [], []
    for alloc in nc.m.functions[0].allocations:
        if not isinstance(alloc, mybir.MemoryLocationSet):
            continue
        name = alloc.memorylocations[0].name
        if alloc.kind == "ExternalInput":
            if name != partition_name:
                in_names.append(name)
        elif alloc.kind == "ExternalOutput":
            np_dt = mybir.dt.np(alloc.dtype)
            out_names.append(name)
            out_avals.append(
                jax.core.ShapedArray(tuple(alloc.tensor_shape), np_dt))
            zero_outs.append(
                np.zeros(tuple(alloc.tensor_shape), np_dt))

    n_params = len(in_names)
    n_outs = len(out_avals)
    all_in_names = in_names + out_names
    if partition_name is not None:
        all_in_names = all_in_names + [partition_name]

    def _body(*args):
        operands = list(args)
        if partition_name is not None:
            operands.append(bass2jax.partition_id_tensor())
        outs = bass2jax._bass_exec_p.bind(
            *operands,
            out_avals=tuple(out_avals),
            in_names=tuple(all_in_names),
            out_names=tuple(out_names),
            lowering_input_output_aliases=(),
            sim_require_finite=True,
            sim_require_nnan=True,
            nc=nc,
        )
        return tuple(outs)

    devices = jax.devices()[:NCORES]
    mesh = Mesh(np.asarray(devices), ("core",))
    in_specs = (PartitionSpec("core"),) * (n_params + n_outs)
    out_specs = (PartitionSpec("core"),) * n_outs
    sharded = jax.jit(
        shard_map(_body, mesh=mesh, in_specs=in_specs, out_specs=out_specs,
                  check_rep=False),
        keep_unused=True,
    )

    concat_in = [
        np.concatenate([in_maps[c][nm] for c in range(NCORES)], axis=0)
        for nm in in_names
    ]
    concat_zeros = [
        np.zeros((NCORES * z.shape[0], *z.shape[1:]), z.dtype)
        for z in zero_outs
    ]
    from jax.sharding import NamedSharding
    sharding = NamedSharding(mesh, PartitionSpec("core"))
    args = [jax.device_put(a, sharding)
            for a in [*concat_in, *concat_zeros]]
    jax.block_until_ready(args)

    def run():
        out = sharded(*args)
        jax.block_until_ready(out)
        return out

    def run_queued(n):
        # queue n executions back-to-back, block once: amortizes the
        # multi-ms axon dispatch overhead so the marginal cost per exec
        # approaches true device time
        o = None
        t0 = time.perf_counter()
        for _ in range(n):
            o = sharded(*args)
        jax.block_until_ready(o)
        return time.perf_counter() - t0

    run.queued = run_queued
    return run, out_names, out_avals


def kernel(**inputs):
    in_maps = _host_prep(inputs)
    nc = _build()
    run, out_names, out_avals = _make_runner(nc, in_maps)

    t0 = time.perf_counter()
    out_arrs = run()  # first call compiles
    t1 = time.perf_counter()

    n_timed = int(os.environ.get("CPLX_TIMED_ITERS", "0"))
    times = []
    for _ in range(n_timed):
        ts = time.perf_counter()
        run()
        times.append(time.perf_counter() - ts)
    marginal = None
    if n_timed:
        # slope over queued batches: subtracts fixed dispatch overhead
        lo, hi = 8, 128
        t_lo = min(run.queued(lo) for _ in range(2))
        t_hi = min(run.queued(hi) for _ in range(2))
        marginal = (t_hi - t_lo) / (hi - lo)
    LAST_TIMING.clear()
    LAST_TIMING.update({
        "first_call_s": t1 - t0,
        "timed_iters_s": times,
        "best_iter_s": min(times) if times else None,
        "marginal_exec_s": marginal,
    })

    yi = out_names.index("y")
    ys = np.asarray(out_arrs[yi]).reshape(NCORES, SQ, D)

    out = np.empty((B, S, D), dtype=np.float32)
    for core in range(NCORES):
        b, half = core // 2, core % 2
        out[b, half * SQ:(half + 1) * SQ, :] = ys[core]
    return out
